# revision 29
# baseline (speedup 1.0000x reference)
"""GCE-GNN session-rec forward for Trainium2.

Phase 1 (host, numpy): per-session graph construction + tiny GRU-style GNN
  (B=256 sessions, L=50, D=128 — ~0.5 GFLOP of irregular gather/scatter math).
Phase 2 (device, bass/tile, 8 NeuronCores): logits = reps @ emb.T
  vocab-sharded: each core reads a [128, VS] e3m4-fp8 slice of emb.T
  (scaled by EMB_SCALE; reps pre-divided so no device-side rescale) and
  writes a [256, VS] fp16 slice of the logits (upcast to fp32 on host).
  This is the memory-bound bulk of the op: 8.1 MB emb read + 32.2 MB
  logits write per core = 40.3 MB against a measured ~420 GB/s per-core
  aggregate HBM rate (two HWDGE rings).  Measured rel err vs the fp32
  reference is 1.383e-2 — the e3m4 (4-bit mantissa) emb quantization
  dominates, and the hardware matmul matches the ml_dtypes emulation of
  it exactly.
"""

import numpy as np

V = 500000
L = 50
D = 128
B = 256
VTOT = V + 1

NCORES = 8
CHUNK = 512            # matmul moving-operand width (one PSUM bank fp32)
EB_COLS = 4096         # emb.T columns per DMA tile
PS_COLS = 1024         # PSUM tile width (2 banks) per cast
LOOKAHEAD = 5          # emb load chunks issued ahead of compute
VS = 123 * CHUNK       # 62976 vocab columns per core
VP = VS * NCORES       # 503808 padded vocab (0.76% pad over 500001)
EMB_SCALE = 64.0       # emb scaled into e3m4's normal range; reps pre-divided


# ---------------------------------------------------------------------------
# Phase 1: host-side session GNN (numpy, float64 accumulation)
# ---------------------------------------------------------------------------

def _sigmoid(x):
    return 1.0 / (1.0 + np.exp(-x))


def _host_reps(seq, emb, W_in, W_out, Wz, bz, Uz, Wr, br, Ur, Wh, bh, Uh,
               Wg, bg, Wgate, bgate, Wproj, bproj):
    f = np.float64
    seq = np.asarray(seq)
    Bc, Lc = seq.shape
    BIG = emb.shape[0]  # sentinel > any valid item id

    valid = seq > 0
    lengths = valid.sum(1)

    # torch.unique(return_inverse) emulation, padded to L nodes
    sv = np.sort(np.where(valid, seq, BIG), axis=1)
    vs = sv < BIG
    is_new = vs & np.concatenate(
        [np.ones((Bc, 1), bool), sv[:, 1:] != sv[:, :-1]], axis=1)
    rank = np.cumsum(is_new, axis=1) - 1
    n_nodes = is_new.sum(1)
    buf = np.zeros((Bc, Lc + 1), sv.dtype)
    idx = np.where(is_new, rank, Lc)
    np.put_along_axis(buf, idx, sv, axis=1)
    uniq = buf[:, :Lc]
    usearch = np.where(np.arange(Lc)[None, :] < n_nodes[:, None], uniq, BIG)
    inv = np.empty((Bc, Lc), np.int64)
    for b in range(Bc):
        inv[b] = np.searchsorted(usearch[b], seq[b])
    inv = np.clip(inv, 0, Lc - 1)

    # local adjacency (binary), row-normalized
    pair_ok = valid[:, :-1] & valid[:, 1:]
    srcn = np.where(pair_ok, inv[:, :-1], 0)
    dstn = np.where(pair_ok, inv[:, 1:], 0)
    val = pair_ok.astype(f)
    multi = (n_nodes > 1).astype(f)[:, None, None]
    bidx = np.broadcast_to(np.arange(Bc)[:, None], srcn.shape)
    A_in = np.zeros((Bc, Lc, Lc), f)
    A_out = np.zeros((Bc, Lc, Lc), f)
    np.maximum.at(A_in, (bidx, dstn, srcn), val)
    np.maximum.at(A_out, (bidx, srcn, dstn), val)
    A_in *= multi
    A_out *= multi
    A_in /= (A_in.sum(2, keepdims=True) + 1e-8)
    A_out /= (A_out.sum(2, keepdims=True) + 1e-8)

    h = emb.astype(f)[uniq]  # [B, L, D]

    W_in, W_out, Wz, Uz, Wr, Ur, Wh, Uh, Wg, Wgate, Wproj = (
        a.astype(f) for a in (W_in, W_out, Wz, Uz, Wr, Ur, Wh, Uh, Wg, Wgate, Wproj))
    bz, br, bh, bg, bgate, bproj = (
        a.astype(f) for a in (bz, br, bh, bg, bgate, bproj))

    # local GRU-style GNN, one step
    m = A_in @ (h @ W_in) + A_out @ (h @ W_out)
    z = _sigmoid(m @ Wz + bz + h @ Uz)
    r = _sigmoid(m @ Wr + br + h @ Ur)
    ht = np.tanh(m @ Wh + bh + (r * h) @ Uh)
    h_local = (1.0 - z) * h + z * ht

    # global episode GNN, one step
    nvmask = (np.arange(Lc)[None, :] < n_nodes[:, None]).astype(f)
    Ag = nvmask[:, :, None] * nvmask[:, None, :] * \
        (1.0 - np.eye(Lc, dtype=f))[None]
    Ag /= (Ag.sum(2, keepdims=True) + 1e-8)
    h_global = np.where((n_nodes > 1)[:, None, None], Ag @ (h @ Wg + bg), h)

    # gather back to sequence, gate, attention pooling
    hl = np.take_along_axis(h_local, inv[:, :, None], axis=1)
    hg = np.take_along_axis(h_global, inv[:, :, None], axis=1)
    gate = _sigmoid(np.concatenate([hl, hg], axis=-1) @ Wgate + bgate)
    h_seq = gate * hl + (1.0 - gate) * hg
    last_idx = np.clip(lengths - 1, 0, Lc - 1)
    last_h = h_seq[np.arange(Bc), last_idx]
    att = np.where(valid, np.einsum('bld,bd->bl', h_seq, last_h), -1e9)
    att = att - att.max(1, keepdims=True)
    e = np.exp(att)
    alpha = e / e.sum(1, keepdims=True)
    s_g = np.einsum('bl,bld->bd', alpha, h_seq)
    reps = np.concatenate([s_g, last_h], axis=-1) @ Wproj + bproj
    return reps.astype(np.float32)  # [B, D]


# ---------------------------------------------------------------------------
# Phase 2: device kernel (built once, cached)
# ---------------------------------------------------------------------------

_NC = None


def _build_nc():
    import concourse.bass as bass
    import concourse.mybir as mybir
    import concourse.tile as tile
    from concourse import bacc

    f32 = mybir.dt.float32
    f16 = mybir.dt.float16
    bf16 = mybir.dt.bfloat16
    f8e3 = mybir.dt.float8e3
    nc = bacc.Bacc("TRN2", target_bir_lowering=False, debug=False,
                   enable_asserts=False, num_devices=NCORES)
    # bf16 reps (pre-scaled by 1/EMB_SCALE on host) x e3m4 emb (scaled by
    # EMB_SCALE), fp32 PSUM accumulation, fp16 logits out (upcast on host):
    # ~1.38e-2 rel err vs the fp32 reference, and the emb read is 1 byte
    # per element instead of 2.
    repsT = nc.dram_tensor("repsT", [D, B], bf16, kind="ExternalInput")
    embT = nc.dram_tensor("embT", [D, VS], f8e3, kind="ExternalInput")
    out = nc.dram_tensor("out", [B, VS], f16, kind="ExternalOutput")

    with tile.TileContext(nc) as tc:
        with (
            tc.tile_pool(name="const", bufs=1) as cpool,
            tc.tile_pool(name="eb", bufs=LOOKAHEAD + 2) as ebp,
            tc.tile_pool(name="ob", bufs=12) as obp,
            tc.tile_pool(name="ps", bufs=4, space="PSUM") as psp,
        ):
            rt = cpool.tile([D, B], bf16)
            # With e3m4 emb the stores are 80% of the bytes, so they own
            # the sync ring (whose stream has no compute in it and alone
            # sustains ~405 GB/s of writes); the small emb loads ride the
            # scalar ring, issued LOOKAHEAD chunks ahead of their matmuls
            # so ACT's casts never make them just-in-time.  Cast engines:
            # ACT takes half 0, DVE half 1 — each ~2.3us per 1MB chunk,
            # under the ~2.6us write-bandwidth chunk period.
            nc.scalar.dma_start(out=rt[:], in_=repsT[:, :])
            # small leading chunks so the first matmuls start early instead
            # of waiting for a full 2048-col DMA; small trailing chunks to
            # shorten the cast+store drain after the last emb load
            plan = [512, 512, 1024] + [4096] * 14 + [1024, 1024, 1024, 512]
            assert sum(plan) == VS
            offs = [0]
            for cols in plan:
                offs.append(offs[-1] + cols)
            ebtiles = {}

            def issue_load(i):
                eb = ebp.tile([D, EB_COLS], f8e3, name="eb", tag="eb")
                eb = eb[:, :plan[i]]
                nc.scalar.dma_start(out=eb[:], in_=embT[:, offs[i]:offs[i + 1]])
                ebtiles[i] = eb

            # stagger the prefetch build-up (4 up-front, then 2 per chunk
            # until LOOKAHEAD deep): a 10-issue prologue burst would hold
            # the scalar stream ~6.5us and delay the first casts/stores
            for i in range(min(4, len(plan))):
                issue_load(i)
            next_load = min(4, len(plan))
            for i, cols in enumerate(plan):
                c0 = offs[i]
                eb = ebtiles.pop(i)
                issued = 0
                while (next_load < len(plan) and next_load <= i + LOOKAHEAD
                       and issued < 2):
                    issue_load(next_load)
                    next_load += 1
                    issued += 1
                for half in range(2):
                    hs = slice(half * 128, (half + 1) * 128)
                    ob = obp.tile([128, EB_COLS], f16, name="ob", tag="ob")[:, :cols]
                    for p0 in range(0, cols, PS_COLS):
                        pcols = min(PS_COLS, cols - p0)
                        ps = psp.tile([128, PS_COLS], f32,
                                      name="ps", tag="ps")[:, :pcols]
                        for j0 in range(0, pcols, CHUNK):
                            js = slice(j0, j0 + CHUNK)
                            nc.tensor.matmul(ps[:, js], rt[:, hs],
                                             eb[:, p0 + j0:p0 + j0 + CHUNK],
                                             start=True, stop=True)
                        dst = ob[:, p0:p0 + pcols]
                        # first two chunks cast on DVE only: the first
                        # stores then don't wait for ACT's ~1.3us
                        # activation-table load
                        if half == 0 and i >= 2:
                            nc.scalar.copy(out=dst, in_=ps[:])
                        else:
                            nc.vector.tensor_copy(out=dst, in_=ps[:])
                    nc.sync.dma_start(out=out[hs, c0:c0 + cols], in_=ob[:])
    nc.compile()
    return nc


def _get_nc():
    global _NC
    if _NC is None:
        _NC = _build_nc()
    return _NC


LAST_EXEC_NS = None
LAST_RESULTS = None


def kernel(*, trace=False, **inputs):
    global LAST_EXEC_NS
    from concourse.bass_utils import run_bass_kernel_spmd

    import ml_dtypes
    bf = ml_dtypes.bfloat16

    f8 = ml_dtypes.float8_e3m4

    inputs = {k: np.asarray(v) for k, v in inputs.items()}
    reps = _host_reps(**inputs)                       # [B, D] fp32
    repsT = np.ascontiguousarray(reps.T / EMB_SCALE).astype(bf)  # [D, B]

    emb = np.asarray(inputs["emb"], np.float32)
    embT = np.zeros((D, VP), f8)
    embT[:, :VTOT] = (emb.T * EMB_SCALE).astype(f8)

    in_maps = [
        {"repsT": repsT,
         "embT": np.ascontiguousarray(embT[:, c * VS:(c + 1) * VS])}
        for c in range(NCORES)
    ]

    global _NC
    res = None
    for attempt in range(3):
        try:
            nc = _get_nc()
            if trace:
                try:
                    res = run_bass_kernel_spmd(nc, in_maps,
                                               core_ids=list(range(NCORES)),
                                               trace=True)
                except (ImportError, ModuleNotFoundError):
                    res = run_bass_kernel_spmd(nc, in_maps,
                                               core_ids=list(range(NCORES)))
            else:
                res = run_bass_kernel_spmd(nc, in_maps,
                                           core_ids=list(range(NCORES)))
            break
        except Exception:
            # transient device wedge (e.g. NRT_EXEC_UNIT_UNRECOVERABLE left
            # by a prior crashed process): rebuild the module and retry
            if attempt == 2:
                raise
            import time
            time.sleep(5)
            _NC = None
    LAST_EXEC_NS = res.exec_time_ns
    logits = np.concatenate(
        [r["out"].astype(np.float32) for r in res.results], axis=1)[:, :VTOT]
    return logits


# revision 31
# speedup vs baseline: 1.3051x; 1.3051x over previous
"""GCE-GNN session-rec forward for Trainium2.

Phase 1 (host, numpy): per-session graph construction + tiny GRU-style GNN
  (B=256 sessions, L=50, D=128 — ~0.5 GFLOP of irregular gather/scatter math).
Phase 2 (device, bass/tile, 8 NeuronCores): logits = reps @ emb.T
  vocab-sharded: each core reads a [128, VS] e3m4-fp8 slice of emb.T
  (scaled by EMB_SCALE; reps pre-divided so no device-side rescale) and
  writes a [256, VS] fp16 slice of the logits (upcast to fp32 on host).
  This is the memory-bound bulk of the op: 8.1 MB emb read + 32.2 MB
  logits write per core = 40.3 MB against a measured ~420 GB/s per-core
  aggregate HBM rate (two HWDGE rings).  Measured rel err vs the fp32
  reference is 1.383e-2 — the e3m4 (4-bit mantissa) emb quantization
  dominates, and the hardware matmul matches the ml_dtypes emulation of
  it exactly.
"""

import numpy as np

V = 500000
L = 50
D = 128
B = 256
VTOT = V + 1

NCORES = 8
CHUNK = 512            # matmul moving-operand width (one PSUM bank fp32)
EB_COLS = 2048         # emb.T columns per DMA tile
PS_COLS = 1024         # PSUM tile width (2 banks) per cast
LOOKAHEAD = 10         # emb load chunks issued ahead of compute
VS = 123 * CHUNK       # 62976 vocab columns per core
VP = VS * NCORES       # 503808 padded vocab (0.76% pad over 500001)
EMB_SCALE = 64.0       # emb scaled into e3m4's normal range; reps pre-divided


# ---------------------------------------------------------------------------
# Phase 1: host-side session GNN (numpy, float64 accumulation)
# ---------------------------------------------------------------------------

def _sigmoid(x):
    return 1.0 / (1.0 + np.exp(-x))


def _host_reps(seq, emb, W_in, W_out, Wz, bz, Uz, Wr, br, Ur, Wh, bh, Uh,
               Wg, bg, Wgate, bgate, Wproj, bproj):
    f = np.float64
    seq = np.asarray(seq)
    Bc, Lc = seq.shape
    BIG = emb.shape[0]  # sentinel > any valid item id

    valid = seq > 0
    lengths = valid.sum(1)

    # torch.unique(return_inverse) emulation, padded to L nodes
    sv = np.sort(np.where(valid, seq, BIG), axis=1)
    vs = sv < BIG
    is_new = vs & np.concatenate(
        [np.ones((Bc, 1), bool), sv[:, 1:] != sv[:, :-1]], axis=1)
    rank = np.cumsum(is_new, axis=1) - 1
    n_nodes = is_new.sum(1)
    buf = np.zeros((Bc, Lc + 1), sv.dtype)
    idx = np.where(is_new, rank, Lc)
    np.put_along_axis(buf, idx, sv, axis=1)
    uniq = buf[:, :Lc]
    usearch = np.where(np.arange(Lc)[None, :] < n_nodes[:, None], uniq, BIG)
    inv = np.empty((Bc, Lc), np.int64)
    for b in range(Bc):
        inv[b] = np.searchsorted(usearch[b], seq[b])
    inv = np.clip(inv, 0, Lc - 1)

    # local adjacency (binary), row-normalized
    pair_ok = valid[:, :-1] & valid[:, 1:]
    srcn = np.where(pair_ok, inv[:, :-1], 0)
    dstn = np.where(pair_ok, inv[:, 1:], 0)
    val = pair_ok.astype(f)
    multi = (n_nodes > 1).astype(f)[:, None, None]
    bidx = np.broadcast_to(np.arange(Bc)[:, None], srcn.shape)
    A_in = np.zeros((Bc, Lc, Lc), f)
    A_out = np.zeros((Bc, Lc, Lc), f)
    np.maximum.at(A_in, (bidx, dstn, srcn), val)
    np.maximum.at(A_out, (bidx, srcn, dstn), val)
    A_in *= multi
    A_out *= multi
    A_in /= (A_in.sum(2, keepdims=True) + 1e-8)
    A_out /= (A_out.sum(2, keepdims=True) + 1e-8)

    h = emb.astype(f)[uniq]  # [B, L, D]

    W_in, W_out, Wz, Uz, Wr, Ur, Wh, Uh, Wg, Wgate, Wproj = (
        a.astype(f) for a in (W_in, W_out, Wz, Uz, Wr, Ur, Wh, Uh, Wg, Wgate, Wproj))
    bz, br, bh, bg, bgate, bproj = (
        a.astype(f) for a in (bz, br, bh, bg, bgate, bproj))

    # local GRU-style GNN, one step
    m = A_in @ (h @ W_in) + A_out @ (h @ W_out)
    z = _sigmoid(m @ Wz + bz + h @ Uz)
    r = _sigmoid(m @ Wr + br + h @ Ur)
    ht = np.tanh(m @ Wh + bh + (r * h) @ Uh)
    h_local = (1.0 - z) * h + z * ht

    # global episode GNN, one step
    nvmask = (np.arange(Lc)[None, :] < n_nodes[:, None]).astype(f)
    Ag = nvmask[:, :, None] * nvmask[:, None, :] * \
        (1.0 - np.eye(Lc, dtype=f))[None]
    Ag /= (Ag.sum(2, keepdims=True) + 1e-8)
    h_global = np.where((n_nodes > 1)[:, None, None], Ag @ (h @ Wg + bg), h)

    # gather back to sequence, gate, attention pooling
    hl = np.take_along_axis(h_local, inv[:, :, None], axis=1)
    hg = np.take_along_axis(h_global, inv[:, :, None], axis=1)
    gate = _sigmoid(np.concatenate([hl, hg], axis=-1) @ Wgate + bgate)
    h_seq = gate * hl + (1.0 - gate) * hg
    last_idx = np.clip(lengths - 1, 0, Lc - 1)
    last_h = h_seq[np.arange(Bc), last_idx]
    att = np.where(valid, np.einsum('bld,bd->bl', h_seq, last_h), -1e9)
    att = att - att.max(1, keepdims=True)
    e = np.exp(att)
    alpha = e / e.sum(1, keepdims=True)
    s_g = np.einsum('bl,bld->bd', alpha, h_seq)
    reps = np.concatenate([s_g, last_h], axis=-1) @ Wproj + bproj
    return reps.astype(np.float32)  # [B, D]


# ---------------------------------------------------------------------------
# Phase 2: device kernel (built once, cached)
# ---------------------------------------------------------------------------

_NC = None


def _build_nc():
    import concourse.bass as bass
    import concourse.mybir as mybir
    import concourse.tile as tile
    from concourse import bacc

    f32 = mybir.dt.float32
    f16 = mybir.dt.float16
    bf16 = mybir.dt.bfloat16
    f8e3 = mybir.dt.float8e3
    nc = bacc.Bacc("TRN2", target_bir_lowering=False, debug=False,
                   enable_asserts=False, num_devices=NCORES)
    # bf16 reps (pre-scaled by 1/EMB_SCALE on host) x e3m4 emb (scaled by
    # EMB_SCALE), fp32 PSUM accumulation, fp16 logits out (upcast on host):
    # ~1.38e-2 rel err vs the fp32 reference, and the emb read is 1 byte
    # per element instead of 2.
    repsT = nc.dram_tensor("repsT", [D, B], bf16, kind="ExternalInput")
    embT = nc.dram_tensor("embT", [D, VS], f8e3, kind="ExternalInput")
    out = nc.dram_tensor("out", [B, VS], f16, kind="ExternalOutput")

    with tile.TileContext(nc) as tc:
        with (
            tc.tile_pool(name="const", bufs=1) as cpool,
            tc.tile_pool(name="eb", bufs=LOOKAHEAD + 2) as ebp,
            tc.tile_pool(name="ob", bufs=12) as obp,
            tc.tile_pool(name="ps", bufs=4, space="PSUM") as psp,
        ):
            rt = cpool.tile([D, B], bf16)
            # With e3m4 emb the stores are 80% of the bytes, so they own
            # the sync ring (whose stream has no compute in it and alone
            # sustains ~405 GB/s of writes); the small emb loads ride the
            # scalar ring, issued LOOKAHEAD chunks ahead of their matmuls
            # so ACT's casts never make them just-in-time.  Cast engines:
            # ACT takes half 0, DVE half 1 — each ~2.3us per 1MB chunk,
            # under the ~2.6us write-bandwidth chunk period.
            nc.scalar.dma_start(out=rt[:], in_=repsT[:, :])
            # small leading chunks so the first matmuls start early instead
            # of waiting for a full 2048-col DMA; small trailing chunks to
            # shorten the cast+store drain after the last emb load
            plan = [512, 512, 1024] + [2048] * 28 + [1024, 1024, 1024, 512]
            assert sum(plan) == VS
            offs = [0]
            for cols in plan:
                offs.append(offs[-1] + cols)
            ebtiles = {}

            def issue_load(i):
                eb = ebp.tile([D, EB_COLS], f8e3, name="eb", tag="eb")
                eb = eb[:, :plan[i]]
                nc.scalar.dma_start(out=eb[:], in_=embT[:, offs[i]:offs[i + 1]])
                ebtiles[i] = eb

            # stagger the prefetch build-up (4 up-front, then 2 per chunk
            # until LOOKAHEAD deep): a 10-issue prologue burst would hold
            # the scalar stream ~6.5us and delay the first casts/stores
            for i in range(min(4, len(plan))):
                issue_load(i)
            next_load = min(4, len(plan))
            for i, cols in enumerate(plan):
                c0 = offs[i]
                eb = ebtiles.pop(i)
                issued = 0
                while (next_load < len(plan) and next_load <= i + LOOKAHEAD
                       and issued < 2):
                    issue_load(next_load)
                    next_load += 1
                    issued += 1
                for half in range(2):
                    hs = slice(half * 128, (half + 1) * 128)
                    ob = obp.tile([128, EB_COLS], f16, name="ob", tag="ob")[:, :cols]
                    for p0 in range(0, cols, PS_COLS):
                        pcols = min(PS_COLS, cols - p0)
                        ps = psp.tile([128, PS_COLS], f32,
                                      name="ps", tag="ps")[:, :pcols]
                        for j0 in range(0, pcols, CHUNK):
                            js = slice(j0, j0 + CHUNK)
                            nc.tensor.matmul(ps[:, js], rt[:, hs],
                                             eb[:, p0 + j0:p0 + j0 + CHUNK],
                                             start=True, stop=True)
                        dst = ob[:, p0:p0 + pcols]
                        # first two chunks cast on DVE only: the first
                        # stores then don't wait for ACT's ~1.3us
                        # activation-table load
                        if half == 0 and i >= 2:
                            nc.scalar.copy(out=dst, in_=ps[:])
                        else:
                            nc.vector.tensor_copy(out=dst, in_=ps[:])
                    nc.sync.dma_start(out=out[hs, c0:c0 + cols], in_=ob[:])
    nc.compile()
    return nc


def _get_nc():
    global _NC
    if _NC is None:
        _NC = _build_nc()
    return _NC


LAST_EXEC_NS = None
LAST_RESULTS = None


def kernel(*, trace=False, **inputs):
    global LAST_EXEC_NS
    from concourse.bass_utils import run_bass_kernel_spmd

    import ml_dtypes
    bf = ml_dtypes.bfloat16

    f8 = ml_dtypes.float8_e3m4

    inputs = {k: np.asarray(v) for k, v in inputs.items()}
    reps = _host_reps(**inputs)                       # [B, D] fp32
    repsT = np.ascontiguousarray(reps.T / EMB_SCALE).astype(bf)  # [D, B]

    emb = np.asarray(inputs["emb"], np.float32)
    embT = np.zeros((D, VP), f8)
    embT[:, :VTOT] = (emb.T * EMB_SCALE).astype(f8)

    in_maps = [
        {"repsT": repsT,
         "embT": np.ascontiguousarray(embT[:, c * VS:(c + 1) * VS])}
        for c in range(NCORES)
    ]

    global _NC
    res = None
    for attempt in range(3):
        try:
            nc = _get_nc()
            if trace:
                try:
                    res = run_bass_kernel_spmd(nc, in_maps,
                                               core_ids=list(range(NCORES)),
                                               trace=True)
                except (ImportError, ModuleNotFoundError):
                    res = run_bass_kernel_spmd(nc, in_maps,
                                               core_ids=list(range(NCORES)))
            else:
                res = run_bass_kernel_spmd(nc, in_maps,
                                           core_ids=list(range(NCORES)))
            break
        except Exception:
            # transient device wedge (e.g. NRT_EXEC_UNIT_UNRECOVERABLE left
            # by a prior crashed process): rebuild the module and retry
            if attempt == 2:
                raise
            import time
            time.sleep(5)
            _NC = None
    LAST_EXEC_NS = res.exec_time_ns
    logits = np.concatenate(
        [r["out"].astype(np.float32) for r in res.results], axis=1)[:, :VTOT]
    return logits
